# revision 11
# baseline (speedup 1.0000x reference)
"""Trainium2 Bass kernel for nn_BaseRuleLearner (compact pair-packed design).

Math (per batch element b, reference semantics):
  UM[b,i,v,l]      = sum_e U[b,l,e]  * ru[i,v,e]
  BM[b,i,n,m,j,k]  = sum_e Bf[b,j,k,e] * rb[i,n,m,e]
  scores[b,i,p]    = sum_v UM[b,i,v,perm[p,v]]
                   + sum_{n,m} BM[b,i,n,m,perm[p,n],perm[p,m]]
  merged[b,i]      = min_p scores[b,i,p]
  out[b,:]         = softmax_i(merged) @ one_hot([0,0,1,1])

Packing (pure data parallel over B across 8 cores, BC=512 b/core):

Offdiag: the 6 ordered (n,m) n!=m gather terms pair up: for unordered
variable pair u={n<m} and unordered object pair {j<k} (28 pairs "jp"),
  CM[b,i,u,d,jp] = BM[b,i,n,m,j',k'] + BM[b,i,m,n,k',j']
with (j',k') = (j,k) if d==0 else (k,j).  Input column (jp,b) stacks
Bf[b,j,k,:] (kappa 0:64) and Bf[b,k,j,:] (kappa 64:128), so one k=128
matmul per jp computes all 24 = (ud,i) outputs; weight wb[128,32]
(24 real cols + 8 zero pad).  Each permutation p then needs only 3
offdiag terms (one per u) instead of 6.

Diag+unary fold: column (l,b) stacks U[b,l,:] and Bf[b,l,l,:];
weight wu2 col (v,i) stacks ru[i,v,:] and rb[i,v,v,:], so
  U2M[b,i,v,l] = UM[b,i,v,l] + BM[b,i,v,v,l,l]
covers unary + diagonal binary terms in one k=128 matmul per l.

PSUM packing: 3 stage-1 outputs (32 rows each: (ud,i), ud zero-padded
to 8) per bank via matmul tile_position col offsets {0,32,64} (96 is
quadrant 3 = unusable); one [96,512] evac copy per bank.  Slots that
no real matmul writes get dummy matmuls so no pre-kernel PSUM NaN can
reach stage-2.

Assembly (i -> columns, (l,ud,jp) -> k-rows) goes through DRAM,
where the partition-boundary reinterpretation is free (SBUF->SBUF
DMAs with >=2 partition dims on the source scramble data - HW
descriptor-pairing bug): hop1 scatters sg[96, (g,b)] -> scratch
DRAM[g, p*512+b] (trivial APs); hop2 reloads scratch viewed flat as
[24g+lud, (i,b)] -> qt (plain copy).  RAW through DRAM is not
dep-tracked, so hop2 gets a forced dep via set_after_insts.
k-chunks: chunk0 [120 rows] = jp 0..14, chunk1 [120] = jp 15..27
(+junk), chunk2 [72] = unary' (v padded to 8); ud padded to 8 keeps
the flat view affine.
Stage-2: psum[128 b, 336 p] accumulated over 3 matmuls vs G chunks
(0/1 gather matrix; junk k-rows have all-zero G rows).  Fused DVE
tensor_tensor_reduce does min(168 vs 168) + reduce in one pass.
"""

import itertools
import numpy as np

B, O, E = 4096, 8, 64
I, V = 4, 3
P = 336
N_CORES = 8
BC = B // N_CORES            # 512 batch per core
NP = 28                      # unordered offdiag object pairs
NBT = BC // 128              # b-tiles per core (4)
NCH = 4                      # ab input DMA chunks (7 jp each)

_PERM = np.array(list(itertools.permutations(range(O), V)), dtype=np.int32)
_PAIRS = [(j, k) for j in range(O) for k in range(j + 1, O)]
_PIDX = np.full((O, O), -1, np.int32)
for _idx, (_j, _k) in enumerate(_PAIRS):
    _PIDX[_j, _k] = _idx
_PAIRS3 = [(0, 1), (0, 2), (1, 2)]

_CACHED = {}


def _build_g():
    """G0[120,P] (jp 0..14: row 24*(jp//3) + 8*(jp%3) + ud),
    G1[120,P] (jp 15..26 same with jp-15; jp27: row 96+ud),
    G2[72,P] (unary': row 24*h + 8*ms + v, l = 3h+ms)."""
    g0 = np.zeros((120, P), np.float32)
    g1 = np.zeros((120, P), np.float32)
    g2 = np.zeros((72, P), np.float32)
    for u, (n, m) in enumerate(_PAIRS3):
        a = _PERM[:, n]
        c = _PERM[:, m]
        j = np.minimum(a, c)
        k = np.maximum(a, c)
        jp = _PIDX[j, k]
        d = (a > c).astype(np.int32)
        ud = u * 2 + d
        for p in range(P):
            jpp = int(jp[p])
            udp = int(ud[p])
            if jpp < 15:
                g0[24 * (jpp // 3) + 8 * (jpp % 3) + udp, p] = 1.0
            elif jpp < 27:
                g1[24 * ((jpp - 15) // 3) + 8 * ((jpp - 15) % 3) + udp, p] = 1.0
            else:
                g1[96 + udp, p] = 1.0
    for v in range(V):
        for p in range(P):
            l = int(_PERM[p, v])
            h = l // 3 if l < 6 else 2
            ms = l - 3 * h
            g2[24 * h + 8 * ms + v, p] = 1.0
    return g0, g1, g2


def _build_module():
    import concourse.tile as tile
    from concourse import bacc, mybir

    FP = mybir.dt.float32
    BF = mybir.dt.bfloat16
    F16 = mybir.dt.float16
    MIN = mybir.AluOpType.min
    nc = bacc.Bacc("TRN2", target_bir_lowering=False, debug=False)

    ab = nc.dram_tensor("ab", [128, NP * BC], BF, kind="ExternalInput")
    au = nc.dram_tensor("au", [128, O * BC], BF, kind="ExternalInput")
    wg = nc.dram_tensor("wg", [128, 64 + 3 * P], BF, kind="ExternalInput")
    out = nc.dram_tensor("out", [BC, 4], FP, kind="ExternalOutput")
    scrA = nc.dram_tensor("scrA", [5, 96 * BC], BF, kind="Internal")
    scrB = nc.dram_tensor("scrB", [5, 96 * BC], BF, kind="Internal")
    scrU = nc.dram_tensor("scrU", [3, 96 * BC], BF, kind="Internal")

    CPJ = NP // NCH          # jp per input chunk (7)

    with tile.TileContext(nc) as tc:
        with (
            tc.tile_pool(name="wpool", bufs=1) as wpool,
            tc.tile_pool(name="mpool", bufs=2) as mpool,
            tc.tile_pool(name="psb", bufs=3, space="PSUM") as psb,
            tc.tile_pool(name="psu", bufs=2, space="PSUM") as psu,
            tc.tile_pool(name="pss", bufs=3, space="PSUM") as pss,
        ):
            # ---- inputs ----
            wg_sb = wpool.tile([128, 64 + 3 * P], BF, tag="wg")
            nc.sync.dma_start(wg_sb[:], wg.ap()[:])
            au_sb = wpool.tile([128, O * BC], BF, tag="au")
            nc.sync.dma_start(au_sb[:], au.ap()[:])
            ab_sb = []
            for c in range(NCH):
                t = wpool.tile([128, CPJ * BC], BF, tag=f"ab{c}", name=f"ab{c}")
                nc.sync.dma_start(
                    t[:], ab.ap()[:, c * CPJ * BC : (c + 1) * CPJ * BC]
                )
                ab_sb.append(t)
            wb_sb = wg_sb[:, 0:32]
            wu_sb = wg_sb[:, 32:64]
            g0_sb = wg_sb[0:120, 64 : 64 + P]
            g1_sb = wg_sb[0:120, 64 + P : 64 + 2 * P]
            g2_sb = wg_sb[0:72, 64 + 2 * P : 64 + 3 * P]

            qt0 = wpool.tile([120, I * BC], BF, tag="qt0")
            qt1 = wpool.tile([120, I * BC], BF, tag="qt1")
            qtu = wpool.tile([72, I * BC], BF, tag="qtu")
            sgu = wpool.tile([96, 3 * BC], BF, tag="sgu")
            sgA = wpool.tile([96, 5 * BC], BF, tag="sgA")
            sgB = wpool.tile([96, 5 * BC], BF, tag="sgB")

            def evac(dst, src):
                nc.vector.tensor_copy(dst, src)

            nasm = [0]

            def assemble(qtc, blk, sgt, cg, scrt):
                """Per-group: sg col-block -> DRAM scratch row (plain)
                -> qt row-block (flat reload); forced RAW dep."""
                row = scrt.ap()[cg : cg + 1, :]
                h1dst = row.rearrange("o (p b) -> (o p) b", p=96)
                eng = nc.sync if nasm[0] % 2 == 0 else nc.scalar
                h1 = eng.dma_start(h1dst, sgt[:, cg * BC : (cg + 1) * BC])
                tc.dep_state.set_after_insts(qtc.tensor.name, h1.ins)
                h2src = row.rearrange("o (r c) -> (o r) c", c=I * BC)
                eng.dma_start(qtc[24 * blk : 24 * blk + 24, :], h2src)
                nasm[0] += 1

            # ---- stage-1 unary' (runs first: warms the PE) ----
            for h in range(3):
                pu = psu.tile([96, BC], FP, tag="pu")
                for ms in range(3):
                    l = 3 * h + ms
                    rl = min(l, O - 1)   # h2 ms2 = dummy slot init (reuse l=7)
                    nc.tensor.matmul(
                        pu[32 * ms : 32 * ms + 32, :],
                        wu_sb,
                        au_sb[:, rl * BC : (rl + 1) * BC],
                        start=True,
                        stop=True,
                    )
                evac(sgu[:, h * BC : (h + 1) * BC], pu[:])
                assemble(qtu, h, sgu, h, scrU)

            # ---- stage-1 offdiag: groups of 3 jp per psum bank ----
            for g in range(10):
                pb = psb.tile([96, BC], FP, tag="pb")
                for l in range(3):
                    jp = min(3 * g + l, NP - 1)   # g9 l1/l2 = dummy (jp27)
                    c = jp // CPJ
                    off = (jp - c * CPJ) * BC
                    nc.tensor.matmul(
                        pb[32 * l : 32 * l + 32, :],
                        wb_sb,
                        ab_sb[c][:, off : off + BC],
                        start=True,
                        stop=True,
                    )
                sgt = sgA if g < 5 else sgB
                scrt = scrA if g < 5 else scrB
                cg = g if g < 5 else g - 5
                evac(sgt[:, cg * BC : (cg + 1) * BC], pb[:])
                assemble(qt0 if g < 5 else qt1, cg, sgt, cg, scrt)

            # ---- stage-2: scores, min, softmax ----
            fin = mpool.tile([128, 4 * NBT], FP, tag="fin", bufs=1)
            for bt in range(NBT):
                merged = mpool.tile([128, 4], FP, tag="m")
                for i in range(I):
                    sc = pss.tile([128, P], FP, tag="sc")
                    col = i * BC + bt * 128
                    nc.tensor.matmul(
                        sc[:], qt0[:, col : col + 128], g0_sb,
                        start=True, stop=False,
                    )
                    nc.tensor.matmul(
                        sc[:], qt1[:, col : col + 128], g1_sb,
                        start=False, stop=False,
                    )
                    nc.tensor.matmul(
                        sc[:], qtu[:, col : col + 128], g2_sb,
                        start=False, stop=True,
                    )
                    if (bt * I + i) % 2 == 0:
                        nc.vector.tensor_reduce(
                            merged[:, i : i + 1], sc[:],
                            axis=mybir.AxisListType.X, op=MIN,
                        )
                    else:
                        sch = mpool.tile([128, P], F16, tag="sch")
                        nc.scalar.copy(sch[:], sc[:])
                        m16 = mpool.tile([128, 1], F16, tag="m16")
                        nc.vector.tensor_reduce(
                            m16[:], sch[:],
                            axis=mybir.AxisListType.X, op=MIN,
                        )
                        nc.gpsimd.tensor_copy(
                            merged[:, i : i + 1], m16[:]
                        )
                mx = mpool.tile([128, 1], FP, tag="mx")
                nc.vector.tensor_reduce(
                    mx[:], merged[:], axis=mybir.AxisListType.X,
                    op=mybir.AluOpType.max,
                )
                sh = mpool.tile([128, 4], FP, tag="sh")
                nc.gpsimd.tensor_scalar_sub(sh[:], merged[:], mx[:])
                ex = mpool.tile([128, 4], FP, tag="ex")
                sm = mpool.tile([128, 1], FP, tag="sm")
                nc.scalar.activation(
                    ex[:], sh[:], mybir.ActivationFunctionType.Exp, accum_out=sm[:]
                )
                rc = mpool.tile([128, 1], FP, tag="rc")
                nc.vector.reciprocal(rc[:], sm[:])
                pr = mpool.tile([128, 4], FP, tag="pr")
                nc.gpsimd.tensor_scalar_mul(pr[:], ex[:], rc[:])
                pr3 = pr[:].rearrange("p (a b) -> p a b", b=2)
                nc.gpsimd.tensor_add(
                    fin[:, bt * 4 : bt * 4 + 2], pr3[:, :, 0], pr3[:, :, 1]
                )
                nc.gpsimd.memset(fin[:, bt * 4 + 2 : bt * 4 + 4], 0.0)
            outv = out.ap().rearrange("(a p) m -> p a m", p=128)
            nc.sync.dma_start(outv, fin[:].rearrange("p (a m) -> p a m", a=NBT))

    nc.compile()
    return nc


def _get_module():
    if "nc" not in _CACHED:
        _CACHED["nc"] = _build_module()
    return _CACHED["nc"]


def _host_inputs(unary_feats, binary_feats, rule_unary, rule_binary):
    import ml_dtypes

    bf16 = ml_dtypes.bfloat16
    uf = np.asarray(unary_feats, dtype=np.float32).astype(bf16)
    bf = np.asarray(binary_feats, dtype=np.float32).astype(bf16)
    ru = np.asarray(rule_unary, dtype=np.float32)
    rb = np.asarray(rule_binary, dtype=np.float32)

    wb = np.zeros((128, 32), np.float32)
    for u, (n, m) in enumerate(_PAIRS3):
        for d in range(2):
            for i in range(I):
                col = (u * 2 + d) * 4 + i
                fst, snd = ((n, m), (m, n)) if d == 0 else ((m, n), (n, m))
                wb[0:64, col] = rb[i, fst[0], fst[1], :]
                wb[64:128, col] = rb[i, snd[0], snd[1], :]
    wu2 = np.zeros((128, 32), np.float32)
    for v in range(V):
        for i in range(I):
            wu2[0:64, v * 4 + i] = ru[i, v, :]
            wu2[64:128, v * 4 + i] = rb[i, v, v, :]
    g0, g1, g2 = _build_g()
    wgm = np.zeros((128, 64 + 3 * P), np.float32)
    wgm[:, 0:32] = wb
    wgm[:, 32:64] = wu2
    wgm[0:120, 64 : 64 + P] = g0
    wgm[0:120, 64 + P : 64 + 2 * P] = g1
    wgm[0:72, 64 + 2 * P : 64 + 3 * P] = g2
    wgm = wgm.astype(bf16)

    J = np.array([p[0] for p in _PAIRS])
    K = np.array([p[1] for p in _PAIRS])
    dia = np.arange(O)
    in_maps = []
    for c in range(N_CORES):
        bfc = bf[c * BC : (c + 1) * BC]                    # [BC, O, O, E]
        x0 = bfc.transpose(1, 2, 3, 0)                     # [j, k, e, b]
        up = x0[J, K]                                      # [28, E, BC]
        dn = x0[K, J]
        abm = np.ascontiguousarray(
            np.concatenate([up, dn], axis=1).transpose(1, 0, 2)
        ).reshape(128, NP * BC)
        ufc = uf[c * BC : (c + 1) * BC]                    # [BC, O, E]
        ut = ufc.transpose(1, 2, 0)                        # [l, e, b]
        dg = bfc[:, dia, dia, :].transpose(1, 2, 0)        # [l, e, b]
        aum = np.ascontiguousarray(
            np.concatenate([ut, dg], axis=1).transpose(1, 0, 2)
        ).reshape(128, O * BC)
        in_maps.append({"ab": abm, "au": aum, "wg": wgm})
    return in_maps


TRACE = False  # set True (e.g. from test.py) to capture an NTFF profile


def kernel(unary_feats, binary_feats, rule_unary, rule_binary):
    from concourse.bass_utils import run_bass_kernel_spmd

    nc = _get_module()
    in_maps = _host_inputs(unary_feats, binary_feats, rule_unary, rule_binary)
    res = run_bass_kernel_spmd(
        nc, in_maps, core_ids=list(range(N_CORES)), trace=TRACE
    )
    _CACHED["last_results"] = res
    return np.concatenate(
        [res.results[c]["out"] for c in range(N_CORES)], axis=0
    )


# revision 15
# speedup vs baseline: 1.1619x; 1.1619x over previous
"""Trainium2 Bass kernel for nn_BaseRuleLearner (compact pair-packed design).

Math (per batch element b, reference semantics):
  UM[b,i,v,l]      = sum_e U[b,l,e]  * ru[i,v,e]
  BM[b,i,n,m,j,k]  = sum_e Bf[b,j,k,e] * rb[i,n,m,e]
  scores[b,i,p]    = sum_v UM[b,i,v,perm[p,v]]
                   + sum_{n,m} BM[b,i,n,m,perm[p,n],perm[p,m]]
  merged[b,i]      = min_p scores[b,i,p]
  out[b,:]         = softmax_i(merged) @ one_hot([0,0,1,1])

Packing (pure data parallel over B across 8 cores, BC=512 b/core):

Offdiag: the 6 ordered (n,m) n!=m gather terms pair up: for unordered
variable pair u={n<m} and unordered object pair {j<k} (28 pairs "jp"),
  CM[b,i,u,d,jp] = BM[b,i,n,m,j',k'] + BM[b,i,m,n,k',j']
with (j',k') = (j,k) if d==0 else (k,j).  Input column (jp,b) stacks
Bf[b,j,k,:] (kappa 0:64) and Bf[b,k,j,:] (kappa 64:128), so one k=128
matmul per jp computes all 24 = (ud,i) outputs; weight wb[128,32]
(24 real cols + 8 zero pad).  Each permutation p then needs only 3
offdiag terms (one per u) instead of 6.

Diag+unary fold: column (l,b) stacks U[b,l,:] and Bf[b,l,l,:];
weight wu2 col (v,i) stacks ru[i,v,:] and rb[i,v,v,:], so
  U2M[b,i,v,l] = UM[b,i,v,l] + BM[b,i,v,v,l,l]
covers unary + diagonal binary terms in one k=128 matmul per l.

PSUM packing: 3 stage-1 outputs (32 rows each: (ud,i), ud zero-padded
to 8) per bank via matmul tile_position col offsets {0,32,64} (96 is
quadrant 3 = unusable); one [96,512] evac copy per bank.  Slots that
no real matmul writes get dummy matmuls so no pre-kernel PSUM NaN can
reach stage-2.

Assembly (i -> columns, (l,ud,jp) -> k-rows) goes through DRAM,
where the partition-boundary reinterpretation is free (SBUF->SBUF
DMAs with >=2 partition dims on the source scramble data - HW
descriptor-pairing bug): hop1 scatters sg[96, (g,b)] -> scratch
DRAM[g, p*512+b] (trivial APs); hop2 reloads scratch viewed flat as
[24g+lud, (i,b)] -> qt (plain copy).  RAW through DRAM is not
dep-tracked, so hop2 gets a forced dep via set_after_insts.
k-chunks: chunk0 [120 rows] = jp 0..14, chunk1 [120] = jp 15..27
(+junk), chunk2 [72] = unary' (v padded to 8); ud padded to 8 keeps
the flat view affine.
Stage-2: psum[128 b, 336 p] accumulated over 3 matmuls vs G chunks
(0/1 gather matrix; junk k-rows have all-zero G rows).  Fused DVE
tensor_tensor_reduce does min(168 vs 168) + reduce in one pass.
"""

import itertools
import numpy as np

B, O, E = 4096, 8, 64
I, V = 4, 3
P = 336
N_CORES = 8
BC = B // N_CORES            # 512 batch per core
NP = 28                      # unordered offdiag object pairs
NBT = BC // 128              # b-tiles per core (4)
NCH = 4                      # ab input DMA chunks (7 jp each)

_PERM = np.array(list(itertools.permutations(range(O), V)), dtype=np.int32)
_PAIRS = [(j, k) for j in range(O) for k in range(j + 1, O)]
_PIDX = np.full((O, O), -1, np.int32)
for _idx, (_j, _k) in enumerate(_PAIRS):
    _PIDX[_j, _k] = _idx
_PAIRS3 = [(0, 1), (0, 2), (1, 2)]

_CACHED = {}


def _build_g():
    """G0[120,P] (jp 0..14: row 24*(jp//3) + 8*(jp%3) + ud),
    G1[120,P] (jp 15..26 same with jp-15; jp27: row 96+ud),
    G2[72,P] (unary': row 24*h + 8*ms + v, l = 3h+ms)."""
    g0 = np.zeros((120, P), np.float32)
    g1 = np.zeros((120, P), np.float32)
    g2 = np.zeros((72, P), np.float32)
    for u, (n, m) in enumerate(_PAIRS3):
        a = _PERM[:, n]
        c = _PERM[:, m]
        j = np.minimum(a, c)
        k = np.maximum(a, c)
        jp = _PIDX[j, k]
        d = (a > c).astype(np.int32)
        ud = u * 2 + d
        for p in range(P):
            jpp = int(jp[p])
            udp = int(ud[p])
            if jpp < 15:
                g0[24 * (jpp // 3) + 8 * (jpp % 3) + udp, p] = 1.0
            elif jpp < 27:
                g1[24 * ((jpp - 15) // 3) + 8 * ((jpp - 15) % 3) + udp, p] = 1.0
            else:
                g1[96 + udp, p] = 1.0
    for v in range(V):
        for p in range(P):
            l = int(_PERM[p, v])
            h = l // 3 if l < 6 else 2
            ms = l - 3 * h
            g2[24 * h + 8 * ms + v, p] = 1.0
    return g0, g1, g2


def _build_module():
    import concourse.tile as tile
    from concourse import bacc, mybir

    FP = mybir.dt.float32
    BF = mybir.dt.bfloat16
    F16 = mybir.dt.float16
    MIN = mybir.AluOpType.min
    nc = bacc.Bacc("TRN2", target_bir_lowering=False, debug=False)

    ab = nc.dram_tensor("ab", [128, NP * BC], BF, kind="ExternalInput")
    awg = nc.dram_tensor(
        "awg", [128, O * BC + 64 + 3 * P], BF, kind="ExternalInput"
    )
    out = nc.dram_tensor("out", [BC, 4], FP, kind="ExternalOutput")
    scrA = nc.dram_tensor("scrA", [5, 96 * BC], BF, kind="Internal")
    scrB = nc.dram_tensor("scrB", [5, 96 * BC], BF, kind="Internal")
    scrU = nc.dram_tensor("scrU", [3, 96 * BC], BF, kind="Internal")

    CPJ = NP // NCH          # jp per input chunk (7)

    with tile.TileContext(nc) as tc:
        with (
            tc.tile_pool(name="wpool", bufs=1) as wpool,
            tc.tile_pool(name="mpool", bufs=2) as mpool,
            tc.tile_pool(name="psb", bufs=3, space="PSUM") as psb,
            tc.tile_pool(name="psu", bufs=2, space="PSUM") as psu,
            tc.tile_pool(name="pss", bufs=3, space="PSUM") as pss,
        ):
            # ---- inputs ----
            awg_sb = wpool.tile([128, O * BC + 64 + 3 * P], BF, tag="awg")
            nc.sync.dma_start(awg_sb[:], awg.ap()[:])
            au_sb = awg_sb[:, 0 : O * BC]
            wg_sb = awg_sb[:, O * BC : O * BC + 64 + 3 * P]
            ab_sb = []
            for c in range(NCH):
                t = wpool.tile([128, CPJ * BC], BF, tag=f"ab{c}", name=f"ab{c}")
                nc.sync.dma_start(
                    t[:], ab.ap()[:, c * CPJ * BC : (c + 1) * CPJ * BC]
                )
                ab_sb.append(t)
            wb_sb = wg_sb[:, 0:32]
            wu_sb = wg_sb[:, 32:64]
            g0_sb = wg_sb[0:120, 64 : 64 + P]
            g1_sb = wg_sb[0:120, 64 + P : 64 + 2 * P]
            g2_sb = wg_sb[0:72, 64 + 2 * P : 64 + 3 * P]

            qt0 = wpool.tile([120, I * BC], BF, tag="qt0")
            qt1 = wpool.tile([120, I * BC], BF, tag="qt1")
            qtu = wpool.tile([72, I * BC], BF, tag="qtu")
            sgu = wpool.tile([96, 3 * BC], BF, tag="sgu")
            sgA = wpool.tile([96, 5 * BC], BF, tag="sgA")
            sgB = wpool.tile([96, 5 * BC], BF, tag="sgB")

            def evac(dst, src):
                nc.vector.tensor_copy(dst, src)

            def hop1(eng, scrt, g0, ng, sgt):
                """sg col-blocks [g0, g0+ng) -> scratch rows:
                scr[g, p*BC + b] = sg[p, g*BC + b]."""
                dst = scrt.ap()[g0 : g0 + ng, :].rearrange(
                    "g (p b) -> p g b", p=96
                )
                src = sgt[:, g0 * BC : (g0 + ng) * BC].rearrange(
                    "p (g b) -> p g b", g=ng
                )
                return eng.dma_start(dst, src)

            def hop2(eng, qtc, r0, scrt, g0, ng, h1):
                """scratch rows reloaded flat -> qt row block."""
                tc.dep_state.set_after_insts(qtc.tensor.name, h1.ins)
                src = scrt.ap()[g0 : g0 + ng, :].rearrange(
                    "g (r c) -> (g r) c", c=I * BC
                )
                eng.dma_start(qtc[r0 : r0 + 24 * ng, :], src)

            # ---- stage-1 unary' (runs first: warms the PE) ----
            for h in range(3):
                pu = psu.tile([96, BC], FP, tag="pu")
                for ms in range(3):
                    l = 3 * h + ms
                    rl = min(l, O - 1)   # h2 ms2 = dummy slot init (reuse l=7)
                    nc.tensor.matmul(
                        pu[32 * ms : 32 * ms + 32, :],
                        wu_sb,
                        au_sb[:, rl * BC : (rl + 1) * BC],
                        start=True,
                        stop=True,
                    )
                evac(sgu[:, h * BC : (h + 1) * BC], pu[:])
            h1u = hop1(nc.scalar, scrU, 0, 3, sgu)
            hop2(nc.scalar, qtu, 0, scrU, 0, 3, h1u)

            # ---- stage-1 offdiag: groups of 3 jp per psum bank ----
            for g in range(10):
                pb = psb.tile([96, BC], FP, tag="pb")
                for l in range(3):
                    jp = min(3 * g + l, NP - 1)   # g9 l1/l2 = dummy (jp27)
                    c = jp // CPJ
                    off = (jp - c * CPJ) * BC
                    nc.tensor.matmul(
                        pb[32 * l : 32 * l + 32, :],
                        wb_sb,
                        ab_sb[c][:, off : off + BC],
                        start=True,
                        stop=True,
                    )
                sgt = sgA if g < 5 else sgB
                cg = g if g < 5 else g - 5
                evac(sgt[:, cg * BC : (cg + 1) * BC], pb[:])
                if g == 4:
                    h1a = hop1(nc.sync, scrA, 0, 5, sgA)
                    hop2(nc.sync, qt0, 0, scrA, 0, 5, h1a)
                if g == 8:
                    h1b1 = hop1(nc.scalar, scrB, 0, 4, sgB)
                    hop2(nc.scalar, qt1, 0, scrB, 0, 4, h1b1)
                if g == 9:
                    h1b2 = hop1(nc.scalar, scrB, 4, 1, sgB)
                    hop2(nc.scalar, qt1, 96, scrB, 4, 1, h1b2)

            # ---- stage-2: scores, min, softmax ----
            fin = mpool.tile([128, 4 * NBT], FP, tag="fin", bufs=1)
            for bt in range(NBT):
                merged = mpool.tile([128, 4], FP, tag="m")
                for i in range(I):
                    sc = pss.tile([128, P], FP, tag="sc")
                    col = i * BC + bt * 128
                    nc.tensor.matmul(
                        sc[:], qt0[:, col : col + 128], g0_sb,
                        start=True, stop=False,
                    )
                    nc.tensor.matmul(
                        sc[:], qt1[:, col : col + 128], g1_sb,
                        start=False, stop=False,
                    )
                    nc.tensor.matmul(
                        sc[:], qtu[:, col : col + 128], g2_sb,
                        start=False, stop=True,
                    )
                    nc.vector.tensor_reduce(
                        merged[:, i : i + 1], sc[:],
                        axis=mybir.AxisListType.X, op=MIN,
                    )
                mx = mpool.tile([128, 1], FP, tag="mx")
                nc.vector.tensor_reduce(
                    mx[:], merged[:], axis=mybir.AxisListType.X,
                    op=mybir.AluOpType.max,
                )
                sh = mpool.tile([128, 4], FP, tag="sh")
                nc.vector.tensor_scalar_sub(sh[:], merged[:], mx[:])
                ex = mpool.tile([128, 4], FP, tag="ex")
                sm = mpool.tile([128, 1], FP, tag="sm")
                nc.scalar.activation(
                    ex[:], sh[:], mybir.ActivationFunctionType.Exp, accum_out=sm[:]
                )
                rc = mpool.tile([128, 1], FP, tag="rc")
                nc.vector.reciprocal(rc[:], sm[:])
                pr = mpool.tile([128, 4], FP, tag="pr")
                nc.vector.tensor_scalar_mul(pr[:], ex[:], rc[:])
                pr3 = pr[:].rearrange("p (a b) -> p a b", b=2)
                nc.vector.tensor_add(
                    fin[:, bt * 4 : bt * 4 + 2], pr3[:, :, 0], pr3[:, :, 1]
                )
                nc.vector.memset(fin[:, bt * 4 + 2 : bt * 4 + 4], 0.0)
            outv = out.ap().rearrange("(a p) m -> p a m", p=128)
            nc.sync.dma_start(outv, fin[:].rearrange("p (a m) -> p a m", a=NBT))

    nc.compile()
    return nc


def _get_module():
    if "nc" not in _CACHED:
        _CACHED["nc"] = _build_module()
    return _CACHED["nc"]


def _host_inputs(unary_feats, binary_feats, rule_unary, rule_binary):
    import ml_dtypes

    bf16 = ml_dtypes.bfloat16
    uf = np.asarray(unary_feats, dtype=np.float32).astype(bf16)
    bf = np.asarray(binary_feats, dtype=np.float32).astype(bf16)
    ru = np.asarray(rule_unary, dtype=np.float32)
    rb = np.asarray(rule_binary, dtype=np.float32)

    wb = np.zeros((128, 32), np.float32)
    for u, (n, m) in enumerate(_PAIRS3):
        for d in range(2):
            for i in range(I):
                col = (u * 2 + d) * 4 + i
                fst, snd = ((n, m), (m, n)) if d == 0 else ((m, n), (n, m))
                wb[0:64, col] = rb[i, fst[0], fst[1], :]
                wb[64:128, col] = rb[i, snd[0], snd[1], :]
    wu2 = np.zeros((128, 32), np.float32)
    for v in range(V):
        for i in range(I):
            wu2[0:64, v * 4 + i] = ru[i, v, :]
            wu2[64:128, v * 4 + i] = rb[i, v, v, :]
    g0, g1, g2 = _build_g()
    wgm = np.zeros((128, 64 + 3 * P), np.float32)
    wgm[:, 0:32] = wb
    wgm[:, 32:64] = wu2
    wgm[0:120, 64 : 64 + P] = g0
    wgm[0:120, 64 + P : 64 + 2 * P] = g1
    wgm[0:72, 64 + 2 * P : 64 + 3 * P] = g2
    wgm = wgm.astype(bf16)

    J = np.array([p[0] for p in _PAIRS])
    K = np.array([p[1] for p in _PAIRS])
    dia = np.arange(O)
    in_maps = []
    for c in range(N_CORES):
        bfc = bf[c * BC : (c + 1) * BC]                    # [BC, O, O, E]
        x0 = bfc.transpose(1, 2, 3, 0)                     # [j, k, e, b]
        up = x0[J, K]                                      # [28, E, BC]
        dn = x0[K, J]
        abm = np.ascontiguousarray(
            np.concatenate([up, dn], axis=1).transpose(1, 0, 2)
        ).reshape(128, NP * BC)
        ufc = uf[c * BC : (c + 1) * BC]                    # [BC, O, E]
        ut = ufc.transpose(1, 2, 0)                        # [l, e, b]
        dg = bfc[:, dia, dia, :].transpose(1, 2, 0)        # [l, e, b]
        aum = np.ascontiguousarray(
            np.concatenate([ut, dg], axis=1).transpose(1, 0, 2)
        ).reshape(128, O * BC)
        awgm = np.concatenate([aum, wgm], axis=1)
        in_maps.append({"ab": abm, "awg": awgm})
    return in_maps


TRACE = False  # set True (e.g. from test.py) to capture an NTFF profile


def kernel(unary_feats, binary_feats, rule_unary, rule_binary):
    from concourse.bass_utils import run_bass_kernel_spmd

    nc = _get_module()
    in_maps = _host_inputs(unary_feats, binary_feats, rule_unary, rule_binary)
    res = run_bass_kernel_spmd(
        nc, in_maps, core_ids=list(range(N_CORES)), trace=TRACE
    )
    _CACHED["last_results"] = res
    return np.concatenate(
        [res.results[c]["out"] for c in range(N_CORES)], axis=0
    )
